# revision 17
# baseline (speedup 1.0000x reference)
"""Causal multi-head attention (B=4, T=2048, D=1024, 16 heads x 64) on 8 trn2 cores.

Sharding: batch x heads hybrid. Core c owns batch c//2 and heads
(c%2)*8 .. +8 (four packed head-pairs of 128). Each core receives x^T for its
batch ([D, T] bf16) plus its 8 heads' W_Q/W_K (stationary layout) and W_V^T
(moving layout); it computes full causal attention for its heads and writes
z^T plus the softmax denominator row per head ([PAIRS, 2, 65, T] f32). The
host normalizes and re-lays-out.

Device kernel design (per core):
  - Q^T/K^T projections as 8-matmul accumulation units per (pair, 512-col
    quarter); stationary weights, moving x (N=512, full PE rate).
  - V produced directly in [t, h] orientation: stationary x^T[d,t] chunk,
    moving W_V^T (N=512) -> no PE transposes at all; DVE copies slice the
    PSUM result into per-(pair,head) v_aug tiles ([128 kt, 16 chunk, 64 v |
    ones | pad]) whose ones-column yields the softmax denominator for free.
  - scores computed transposed S^T[kt, qt]; the two heads of a pair run as
    concurrent PE row-tiles (K=64 at partition bases 0/64) into the two
    halves of one [128, 1024] f32 PSUM; ONE exp (scale=1/8 fused) per chunk
    covers both heads (diagonal chunks span the dead middle columns; that
    garbage is never read downstream).
  - causal mask applied post-exp with gpsimd affine_select (idle engine).
  - AV is software-pipelined a few chunks behind the scores and accumulates
    z^T + denominator in f32 PSUM.
  - THE SCHEDULING POINT: exp on ScalarE (~166us/core) and matmuls on PE
    (~169us/core) are nearly balanced, and engine queues are FIFO - so the
    emission order interleaves projection/V units between attention chunks
    (ratio-paced + deadline-forced) across ALL four pairs' attention,
    keeping both engines busy instead of serializing phase by phase.
"""

import os
import sys

for _p in ("/opt/trn_rl_repo", "/root/.axon_site/_ro/trn_rl_repo"):
    if os.path.isdir(_p) and _p not in sys.path:
        sys.path.insert(0, _p)

import ml_dtypes
import numpy as np

import concourse.bass as bass
import concourse.mybir as mybir
import concourse.tile as tile
from concourse import bacc
from concourse.bass import ds
from concourse.bass_utils import run_bass_kernel_spmd

B, T, D = 4, 2048, 1024
NH, DH = 16, 64
NCORES = 8
HPB = 8                     # heads per core (batch x head sharding)
PAIRS = HPB // 2            # packed head-pairs per core = 4
H2 = 2 * DH                 # packed pair dim = 128
P = 128
QT = 512                    # query-tile width (psum bank limit for f32 out)
NQ4 = T // QT               # 4 query tiles
NCH = T // P                # 16 key chunks
KD = D // P                 # 8 contraction chunks
F32 = mybir.dt.float32
BF16 = mybir.dt.bfloat16
SCALE = 1.0 / np.sqrt(DH)   # 0.125


def _build(nc, tc, xT_d, w_d, out_d):
    from contextlib import ExitStack

    AF = mybir.ActivationFunctionType
    OP = mybir.AluOpType

    with ExitStack() as ctx:
        ep = ctx.enter_context
        const = ep(tc.tile_pool(name="const", bufs=1))
        xt_pool = ep(tc.tile_pool(name="xt", bufs=1))
        qk_pool = ep(tc.tile_pool(name="qk", bufs=1))
        va_pool = ep(tc.tile_pool(name="va", bufs=1))
        p_pool = ep(tc.tile_pool(name="pp", bufs=8))
        zt_pool = ep(tc.tile_pool(name="zt", bufs=4))
        ps_acc = ep(tc.tile_pool(name="ps_acc", bufs=2, space="PSUM"))
        ps_s = ep(tc.tile_pool(name="ps_s", bufs=2, space="PSUM"))
        ps_z = ep(tc.tile_pool(name="ps_z", bufs=2, space="PSUM"))

        # ---- DMA order matters: pair-0 Q/K weights, then the first x^T
        # quarter, so the first projection unit can start ~1.5us in ----
        w_sb = {}
        for name in ("wq", "wk"):
            w_sb[name] = const.tile(
                [P, PAIRS, KD, H2], BF16, tag=name, name=name
            )
        wv_sb = const.tile([P, KD, HPB * DH], BF16, tag="wv", name="wv_sb")
        xt = [
            xt_pool.tile([P, T], BF16, tag=f"x{k}", name="xt") for k in range(KD)
        ]

        def dma_w_pair(p):
            for name in ("wq", "wk"):
                nc.sync.dma_start(
                    w_sb[name][:, p],
                    w_d[name].rearrange(
                        "p (r c h) -> p r c h", r=PAIRS, c=KD
                    )[:, p],
                )

        dma_w_pair(0)
        for k in range(KD):
            nc.sync.dma_start(
                xt[k][:, ds(0, QT)], xT_d[ds(k * P, P), ds(0, QT)]
            )
        nc.sync.dma_start(wv_sb[:], w_d["wv"].rearrange("p (c h) -> p c h", c=KD))
        dma_w_pair(1)
        for t4 in range(1, NQ4):
            for k in range(KD):
                nc.sync.dma_start(
                    xt[k][:, ds(t4 * QT, QT)], xT_d[ds(k * P, P), ds(t4 * QT, QT)]
                )
            if t4 + 1 < PAIRS:
                dma_w_pair(t4 + 1)

        qt = [
            qk_pool.tile([P, T], BF16, tag=f"qt{p}", name="qt") for p in range(PAIRS)
        ]
        kt = [
            qk_pool.tile([P, T], BF16, tag=f"kt{p}", name="kt") for p in range(PAIRS)
        ]
        # v_aug: [kt, chunk, 64 v-cols | ones col | 63 zeros]
        va = [
            [
                va_pool.tile([P, NCH, P], BF16, tag=f"v{p}{a}", name="va")
                for a in range(2)
            ]
            for p in range(PAIRS)
        ]
        for p in range(PAIRS):
            for a in range(2):
                nc.gpsimd.memset(va[p][a][:, :, DH:P], 0.0)
                nc.gpsimd.memset(va[p][a][:, :, DH : DH + 1], 1.0)

        # ---- background work units (projections + V), emitted between
        # attention chunks by the driver below ----
        # units are emitted in 2-matmul quarters so the pacer can weave
        # projection work between chunks at sub-microsecond granularity
        def qk_unit(name, p, t4):
            dst = qt[p] if name == "wq" else kt[p]
            box = {}

            def emit_quarter(q):
                def emit():
                    if q == 0:
                        box["acc"] = ps_acc.tile(
                            [P, QT], F32, tag="acc", name="acc"
                        )
                    acc = box["acc"]
                    for k in range(2 * q, 2 * q + 2):
                        nc.tensor.matmul(
                            acc[:],
                            w_sb[name][:, p, k, :],
                            xt[k][:, ds(t4 * QT, QT)],
                            start=(k == 0),
                            stop=(k == KD - 1),
                        )
                    if q == 3:
                        nc.vector.tensor_copy(dst[:, ds(t4 * QT, QT)], acc[:])

                return emit

            return [emit_quarter(q) for q in range(4)]

        def v_unit(j):
            box = {}

            def emit_quarter(q):
                def emit():
                    if q == 0:
                        box["pv"] = ps_acc.tile(
                            [P, HPB * DH], F32, tag="acc", name="pv"
                        )
                    pv = box["pv"]
                    for k in range(2 * q, 2 * q + 2):
                        nc.tensor.matmul(
                            pv[:],
                            xt[k][:, ds(j * P, P)],
                            wv_sb[:, k, :],
                            start=(k == 0),
                            stop=(k == KD - 1),
                        )
                    if q == 3:
                        for p in range(PAIRS):
                            for a in range(2):
                                nc.vector.tensor_copy(
                                    va[p][a][:, j, 0:DH],
                                    pv[:, ds((2 * p + a) * DH, DH)],
                                )

                return emit

            return [emit_quarter(q) for q in range(4)]

        # ---- EDF schedule of projection/V units between attention chunks.
        # Tiles run q4-major, pair-minor; each unit gets a chunk-indexed
        # deadline (the chunk before which it must be emitted), and a rate
        # pacer walks the deadline-sorted list early so the PE never takes
        # a multi-unit burst that starves the exp stream. ----
        MPB = QT // P
        tile_start = {}
        cs = 0
        for q4 in range(NQ4):
            for p in range(PAIRS):
                tile_start[(q4, p)] = cs
                cs += (q4 + 1) * MPB
        total_chunks = cs  # 160

        LAG = 4  # chunks the AV matmuls trail the score/exp stream by
        units = []  # (deadline, order, emit_fn) -- one entry per quarter
        for p in range(PAIRS):
            for t4 in range(NQ4):
                # qt quarter q4 is read by every chunk of tile (q4, p);
                # kt quarter t4 is first read at local chunk 4*t4
                for q, fn in enumerate(qk_unit("wq", p, t4)):
                    units.append((tile_start[(t4, p)], (0, q), fn))
                for q, fn in enumerate(qk_unit("wk", p, t4)):
                    units.append(
                        (tile_start[(t4, p)] + t4 * MPB, (1, q), fn)
                    )
        for j in range(NCH):
            q4 = j // MPB
            # v chunk j is first read by the trailing AV of tile (q4, 0)
            dl = tile_start[(q4, 0)] + (j - q4 * MPB) + LAG
            for q, fn in enumerate(v_unit(j)):
                units.append((dl, (2, j, q), fn))
        units.sort(key=lambda u: (u[0], u[1]))
        n_units = len(units)
        uidx = [0]

        def inject_bg(c):
            # deadline-forced, then rate-paced (n_units spread over chunks)
            while uidx[0] < n_units and units[uidx[0]][0] <= c:
                units[uidx[0]][2]()
                uidx[0] += 1
            while (uidx[0] < n_units
                   and uidx[0] * total_chunks < n_units * c):
                units[uidx[0]][2]()
                uidx[0] += 1

        # ---- attention: one global chunk stream (q4-major, pair-minor);
        # the AV matmuls trail the score/exp stream by LAG chunks even
        # across tile boundaries, so the next tile's scores keep the exp
        # pipeline fed while the previous tile's AV+drain runs ----
        zrow = DH + 1
        stream = []
        for q4 in range(NQ4):
            for p in range(PAIRS):
                njs = (q4 + 1) * MPB
                for j in range(njs):
                    stream.append((q4, p, j, njs))

        pend = []  # (q4, p, j, njs, c0, pe) awaiting the AV matmuls
        pz_map = {}

        def emit_av():
            q4, p, jj, njs, cc0, ppe = pend.pop(0)
            if jj == 0:
                pz_map[(q4, p)] = [
                    ps_z.tile([P, QT], F32, tag="z", name="pz") for _ in range(2)
                ]
            pz = pz_map[(q4, p)]
            for a in range(2):
                nc.tensor.matmul(
                    pz[a][:, cc0:QT],
                    va[p][a][:, jj, :],
                    ppe[:, a * QT + cc0 : (a + 1) * QT],
                    start=(jj == 0),
                    stop=(jj == njs - 1),
                    skip_group_check=True,
                )
            if jj == njs - 1:
                for a in range(2):
                    zt_t = zt_pool.tile([zrow, QT], F32, tag="zt", name="zt_t")
                    nc.vector.tensor_copy(zt_t[:], pz[a][0:zrow, :])
                    nc.sync.dma_start(out_d[p, a, :, ds(q4 * QT, QT)], zt_t[:])
                del pz_map[(q4, p)]

        for c, (q4, p, j, njs) in enumerate(stream):
            # trailing AV first: it is always ready (its exp+mask are LAG
            # chunks old) and buys the current chunk's exp dependency time
            if len(pend) > LAG:
                emit_av()
            inject_bg(c)
            rdiag = j - q4 * MPB
            c0 = 0 if rdiag < 0 else rdiag * P
            w_hi = (rdiag + 1) * P if rdiag >= 0 else 0
            nw = QT - c0
            ss = ps_s.tile([P, 2 * QT], F32, tag="s", name="ss")
            pe = p_pool.tile([P, 2 * QT], BF16, tag="p", name="pe")
            for a in range(2):
                hp = ds(a * DH, DH)
                nc.tensor.matmul(
                    ss[:, a * QT + c0 : (a + 1) * QT],
                    kt[p][hp, ds(j * P, P)],
                    qt[p][hp, ds(q4 * QT + c0, nw)],
                    start=True,
                    stop=True,
                )
            # one exp covering both heads' valid column blocks via a
            # strided AP (no read of the dead middle columns)
            nc.scalar.activation(
                pe.rearrange("p (a q) -> p a q", a=2)[:, :, c0:QT],
                ss.rearrange("p (a q) -> p a q", a=2)[:, :, c0:QT],
                AF.Exp,
                scale=float(SCALE),
            )
            if rdiag >= 0:
                # keep iff qt >= kt  <=>  (col - p - 128*rdiag) >= 0
                for a in range(2):
                    nc.gpsimd.affine_select(
                        out=pe[:, a * QT + c0 : a * QT + w_hi],
                        in_=pe[:, a * QT + c0 : a * QT + w_hi],
                        compare_op=OP.is_ge,
                        fill=0.0,
                        base=c0 - rdiag * P,
                        pattern=[[1, w_hi - c0]],
                        channel_multiplier=-1,
                    )
            pend.append((q4, p, j, njs, c0, pe))
        while pend:
            emit_av()

        # drain any remaining units (shouldn't happen)
        while uidx[0] < n_units:
            units[uidx[0]][2]()
            uidx[0] += 1


def build_bass():
    nc = bacc.Bacc(None, target_bir_lowering=False)
    xT_d = nc.declare_dram_parameter("xT", [D, T], BF16, isOutput=False)
    w_d = {
        "wq": nc.declare_dram_parameter(
            "wq", [P, KD * PAIRS * H2], BF16, isOutput=False
        ),
        "wk": nc.declare_dram_parameter(
            "wk", [P, KD * PAIRS * H2], BF16, isOutput=False
        ),
        "wv": nc.declare_dram_parameter(
            "wv", [P, KD * HPB * DH], BF16, isOutput=False
        ),
    }
    out_d = nc.declare_dram_parameter(
        "out", [PAIRS, 2, DH + 1, T], F32, isOutput=True
    )
    with tile.TileContext(nc) as tc:
        _build(nc, tc, xT_d, w_d, out_d)
    nc.compile()
    return nc


_CACHE = {}


def _get_nc():
    if "nc" not in _CACHE:
        _CACHE["nc"] = build_bass()
    return _CACHE["nc"]


def make_in_maps(x, W_K, W_Q, W_V):
    x = np.asarray(x, dtype=np.float32)
    in_maps = []
    for c in range(NCORES):
        b = c // 2
        hb = (c % 2) * HPB
        xT = np.ascontiguousarray(x[b].T).astype(ml_dtypes.bfloat16)

        def stat(w):  # stationary layout for Q/K: [P, PAIRS, KD, H2]
            w = np.asarray(w, dtype=np.float32)
            arr = np.empty((P, PAIRS, KD, H2), np.float32)
            for p in range(PAIRS):
                wp = w[hb + 2 * p : hb + 2 * p + 2].reshape(H2, D).T  # [D, H2]
                arr[:, p, :, :] = wp.reshape(KD, P, H2).transpose(1, 0, 2)
            return np.ascontiguousarray(
                arr.reshape(P, PAIRS * KD * H2)
            ).astype(ml_dtypes.bfloat16)

        def mov(w):  # moving layout for V: [P, KD, HPB*DH]
            w = np.asarray(w, dtype=np.float32)
            wt = w[hb : hb + HPB].reshape(HPB * DH, D).T  # [D, 8*64] head-major
            wt = wt.reshape(KD, P, HPB * DH).transpose(1, 0, 2)
            return np.ascontiguousarray(
                wt.reshape(P, KD * HPB * DH)
            ).astype(ml_dtypes.bfloat16)

        in_maps.append(
            {"xT": xT, "wq": stat(W_Q), "wk": stat(W_K), "wv": mov(W_V)}
        )
    return in_maps


def kernel(x, W_K, W_Q, W_V, _trace=False, _trace_kwargs=None):
    in_maps = make_in_maps(x, W_K, W_Q, W_V)
    res = run_bass_kernel_spmd(
        _get_nc(),
        in_maps,
        list(range(NCORES)),
        trace=_trace,
        **(_trace_kwargs or {}),
    )
    _CACHE["last_results"] = res
    out = np.empty((B, T, NH * DH), np.float32)
    for c in range(NCORES):
        zt = np.asarray(res.results[c]["out"])  # [PAIRS, 2, DH+1, T]
        z = zt[:, :, :DH, :] / zt[:, :, DH : DH + 1, :]
        b = c // 2
        hb = (c % 2) * HPB
        for p in range(PAIRS):
            for a in range(2):
                h = hb + 2 * p + a
                out[b, :, h * DH : (h + 1) * DH] = z[p, a].T
    return out


# revision 19
# speedup vs baseline: 1.2006x; 1.2006x over previous
"""Causal multi-head attention (B=4, T=2048, D=1024, 16 heads x 64) on 8 trn2 cores.

Sharding: batch x heads hybrid. Core c owns batch c//2 and heads
(c%2)*8 .. +8 (four packed head-pairs of 128). Each core receives x^T for its
batch ([D, T] bf16) plus its 8 heads' W_Q/W_K (stationary layout) and W_V^T
(moving layout); it computes full causal attention for its heads and writes
z^T plus the softmax denominator row per head ([PAIRS, 2, 65, T] f32). The
host normalizes and re-lays-out.

Device kernel design (per core):
  - Q^T/K^T projections as 8-matmul accumulation units per (pair, 512-col
    quarter); stationary weights, moving x (N=512, full PE rate).
  - V produced directly in [t, h] orientation: stationary x^T[d,t] chunk,
    moving W_V^T (N=512) -> no PE transposes at all; DVE copies slice the
    PSUM result into per-(pair,head) v_aug tiles ([128 kt, 16 chunk, 64 v |
    ones | pad]) whose ones-column yields the softmax denominator for free.
  - scores computed transposed S^T[kt, qt]; the two heads of a pair run as
    concurrent PE row-tiles (K=64 at partition bases 0/64) into the two
    halves of one [128, 1024] f32 PSUM; ONE exp (scale=1/8 fused) per chunk
    covers both heads (diagonal chunks span the dead middle columns; that
    garbage is never read downstream).
  - causal mask applied post-exp with gpsimd affine_select (idle engine).
  - AV is software-pipelined a few chunks behind the scores and accumulates
    z^T + denominator in f32 PSUM.
  - THE SCHEDULING POINT: exp on ScalarE (~166us/core) and matmuls on PE
    (~169us/core) are nearly balanced, and engine queues are FIFO - so the
    emission order interleaves projection/V units between attention chunks
    (ratio-paced + deadline-forced) across ALL four pairs' attention,
    keeping both engines busy instead of serializing phase by phase.
"""

import os
import sys

for _p in ("/opt/trn_rl_repo", "/root/.axon_site/_ro/trn_rl_repo"):
    if os.path.isdir(_p) and _p not in sys.path:
        sys.path.insert(0, _p)

import ml_dtypes
import numpy as np

import concourse.bass as bass
import concourse.mybir as mybir
import concourse.tile as tile
from concourse import bacc
from concourse.bass import ds
from concourse.bass_utils import run_bass_kernel_spmd

B, T, D = 4, 2048, 1024
NH, DH = 16, 64
NCORES = 8
HPB = 8                     # heads per core (batch x head sharding)
PAIRS = HPB // 2            # packed head-pairs per core = 4
H2 = 2 * DH                 # packed pair dim = 128
P = 128
QT = 512                    # query-tile width (psum bank limit for f32 out)
NQ4 = T // QT               # 4 query tiles
NCH = T // P                # 16 key chunks
KD = D // P                 # 8 contraction chunks
F32 = mybir.dt.float32
BF16 = mybir.dt.bfloat16
SCALE = 1.0 / np.sqrt(DH)   # 0.125


def _build(nc, tc, xT_d, w_d, out_d):
    from contextlib import ExitStack

    AF = mybir.ActivationFunctionType
    OP = mybir.AluOpType

    with ExitStack() as ctx:
        ep = ctx.enter_context
        const = ep(tc.tile_pool(name="const", bufs=1))
        xt_pool = ep(tc.tile_pool(name="xt", bufs=1))
        qk_pool = ep(tc.tile_pool(name="qk", bufs=1))
        va_pool = ep(tc.tile_pool(name="va", bufs=1))
        p_pool = ep(tc.tile_pool(name="pp", bufs=8))
        zt_pool = ep(tc.tile_pool(name="zt", bufs=4))
        ps_acc = ep(tc.tile_pool(name="ps_acc", bufs=2, space="PSUM"))
        ps_s = ep(tc.tile_pool(name="ps_s", bufs=2, space="PSUM"))
        ps_z = ep(tc.tile_pool(name="ps_z", bufs=2, space="PSUM"))

        # ---- DMA order matters: pair-0 Q/K weights, then the first x^T
        # quarter, so the first projection unit can start ~1.5us in ----
        w_sb = {}
        for name in ("wq", "wk"):
            w_sb[name] = const.tile(
                [P, PAIRS, KD, H2], BF16, tag=name, name=name
            )
        wv_sb = const.tile([P, KD, HPB * DH], BF16, tag="wv", name="wv_sb")
        xt = [
            xt_pool.tile([P, T], BF16, tag=f"x{k}", name="xt") for k in range(KD)
        ]

        def dma_w_pair(p):
            for name in ("wq", "wk"):
                nc.sync.dma_start(
                    w_sb[name][:, p],
                    w_d[name].rearrange(
                        "p (r c h) -> p r c h", r=PAIRS, c=KD
                    )[:, p],
                )

        dma_w_pair(0)
        for k in range(KD):
            nc.sync.dma_start(
                xt[k][:, ds(0, QT)], xT_d[ds(k * P, P), ds(0, QT)]
            )
        nc.sync.dma_start(wv_sb[:], w_d["wv"].rearrange("p (c h) -> p c h", c=KD))
        dma_w_pair(1)
        for t4 in range(1, NQ4):
            for k in range(KD):
                nc.sync.dma_start(
                    xt[k][:, ds(t4 * QT, QT)], xT_d[ds(k * P, P), ds(t4 * QT, QT)]
                )
            if t4 + 1 < PAIRS:
                dma_w_pair(t4 + 1)

        qt = [
            qk_pool.tile([P, T], BF16, tag=f"qt{p}", name="qt") for p in range(PAIRS)
        ]
        kt = [
            qk_pool.tile([P, T], BF16, tag=f"kt{p}", name="kt") for p in range(PAIRS)
        ]
        # v_aug: [kt, chunk, 64 v-cols | ones col | 63 zeros]
        va = [
            [
                va_pool.tile([P, NCH, P], BF16, tag=f"v{p}{a}", name="va")
                for a in range(2)
            ]
            for p in range(PAIRS)
        ]
        for p in range(PAIRS):
            for a in range(2):
                nc.gpsimd.memset(va[p][a][:, :, DH:P], 0.0)
                nc.gpsimd.memset(va[p][a][:, :, DH : DH + 1], 1.0)

        # ---- background work units (projections + V), emitted between
        # attention chunks by the driver below ----
        def qk_unit(name, p, t4):
            dst = qt[p] if name == "wq" else kt[p]

            def emit():
                acc = ps_acc.tile([P, QT], F32, tag="acc", name="acc")
                for k in range(KD):
                    nc.tensor.matmul(
                        acc[:],
                        w_sb[name][:, p, k, :],
                        xt[k][:, ds(t4 * QT, QT)],
                        start=(k == 0),
                        stop=(k == KD - 1),
                    )
                nc.vector.tensor_copy(dst[:, ds(t4 * QT, QT)], acc[:])

            return emit

        def v_unit(j):
            def emit():
                pv = ps_acc.tile([P, HPB * DH], F32, tag="acc", name="pv")
                for k in range(KD):
                    nc.tensor.matmul(
                        pv[:],
                        xt[k][:, ds(j * P, P)],
                        wv_sb[:, k, :],
                        start=(k == 0),
                        stop=(k == KD - 1),
                    )
                for p in range(PAIRS):
                    for a in range(2):
                        nc.vector.tensor_copy(
                            va[p][a][:, j, 0:DH], pv[:, ds((2 * p + a) * DH, DH)]
                        )

            return emit

        # ---- EDF schedule of projection/V units between attention chunks.
        # Tiles run q4-major, pair-minor; each unit gets a chunk-indexed
        # deadline (the chunk before which it must be emitted), and a rate
        # pacer walks the deadline-sorted list early so the PE never takes
        # a multi-unit burst that starves the exp stream. ----
        MPB = QT // P
        tile_start = {}
        cs = 0
        for q4 in range(NQ4):
            for p in range(PAIRS):
                tile_start[(q4, p)] = cs
                cs += (q4 + 1) * MPB
        total_chunks = cs  # 160

        LAG = 4  # chunks the AV matmuls trail the score/exp stream by
        units = []  # (deadline, order, emit_fn) -- one entry per quarter
        for p in range(PAIRS):
            for t4 in range(NQ4):
                # qt quarter q4 is read by every chunk of tile (q4, p);
                # kt quarter t4 is first read at local chunk 4*t4
                units.append((tile_start[(t4, p)], 0, qk_unit("wq", p, t4)))
                units.append(
                    (tile_start[(t4, p)] + t4 * MPB, 1, qk_unit("wk", p, t4))
                )
        for j in range(NCH):
            q4 = j // MPB
            # v chunk j is first read by the trailing AV of tile (q4, 0)
            dl = tile_start[(q4, 0)] + (j - q4 * MPB) + LAG
            units.append((dl, 2, v_unit(j)))
        units.sort(key=lambda u: (u[0], u[1]))
        n_units = len(units)
        uidx = [0]

        def inject_bg(c):
            # deadline-forced, then rate-paced (n_units spread over chunks)
            while uidx[0] < n_units and units[uidx[0]][0] <= c:
                units[uidx[0]][2]()
                uidx[0] += 1
            while (uidx[0] < n_units
                   and uidx[0] * total_chunks < n_units * c):
                units[uidx[0]][2]()
                uidx[0] += 1

        # ---- attention: one global chunk stream (q4-major, pair-minor);
        # the AV matmuls trail the score/exp stream by LAG chunks even
        # across tile boundaries, so the next tile's scores keep the exp
        # pipeline fed while the previous tile's AV+drain runs ----
        zrow = DH + 1
        stream = []
        for q4 in range(NQ4):
            for p in range(PAIRS):
                njs = (q4 + 1) * MPB
                for j in range(njs):
                    stream.append((q4, p, j, njs))

        pend = []  # (q4, p, j, njs, c0, pe) awaiting the AV matmuls
        pz_map = {}

        def emit_av():
            q4, p, jj, njs, cc0, ppe = pend.pop(0)
            if jj == 0:
                pz_map[(q4, p)] = [
                    ps_z.tile([P, QT], F32, tag="z", name="pz") for _ in range(2)
                ]
            pz = pz_map[(q4, p)]
            for a in range(2):
                nc.tensor.matmul(
                    pz[a][:, cc0:QT],
                    va[p][a][:, jj, :],
                    ppe[:, a * QT + cc0 : (a + 1) * QT],
                    start=(jj == 0),
                    stop=(jj == njs - 1),
                    skip_group_check=True,
                )
            if jj == njs - 1:
                for a in range(2):
                    zt_t = zt_pool.tile([zrow, QT], F32, tag="zt", name="zt_t")
                    nc.vector.tensor_copy(zt_t[:], pz[a][0:zrow, :])
                    nc.sync.dma_start(out_d[p, a, :, ds(q4 * QT, QT)], zt_t[:])
                del pz_map[(q4, p)]

        for c, (q4, p, j, njs) in enumerate(stream):
            # trailing AV first: it is always ready (its exp+mask are LAG
            # chunks old) and buys the current chunk's exp dependency time
            if len(pend) > LAG:
                emit_av()
            inject_bg(c)
            rdiag = j - q4 * MPB
            c0 = 0 if rdiag < 0 else rdiag * P
            w_hi = (rdiag + 1) * P if rdiag >= 0 else 0
            nw = QT - c0
            ss = ps_s.tile([P, 2 * QT], F32, tag="s", name="ss")
            pe = p_pool.tile([P, 2 * QT], BF16, tag="p", name="pe")
            for a in range(2):
                hp = ds(a * DH, DH)
                nc.tensor.matmul(
                    ss[:, a * QT + c0 : (a + 1) * QT],
                    kt[p][hp, ds(j * P, P)],
                    qt[p][hp, ds(q4 * QT + c0, nw)],
                    start=True,
                    stop=True,
                )
            # one exp covering both heads' valid column blocks via a
            # strided AP (no read of the dead middle columns)
            nc.scalar.activation(
                pe.rearrange("p (a q) -> p a q", a=2)[:, :, c0:QT],
                ss.rearrange("p (a q) -> p a q", a=2)[:, :, c0:QT],
                AF.Exp,
                scale=float(SCALE),
            )
            if rdiag >= 0:
                # keep iff qt >= kt  <=>  (col - p - 128*rdiag) >= 0
                for a in range(2):
                    nc.gpsimd.affine_select(
                        out=pe[:, a * QT + c0 : a * QT + w_hi],
                        in_=pe[:, a * QT + c0 : a * QT + w_hi],
                        compare_op=OP.is_ge,
                        fill=0.0,
                        base=c0 - rdiag * P,
                        pattern=[[1, w_hi - c0]],
                        channel_multiplier=-1,
                    )
            pend.append((q4, p, j, njs, c0, pe))
        while pend:
            emit_av()

        # drain any remaining units (shouldn't happen)
        while uidx[0] < n_units:
            units[uidx[0]][2]()
            uidx[0] += 1


def build_bass():
    nc = bacc.Bacc(None, target_bir_lowering=False)
    xT_d = nc.declare_dram_parameter("xT", [D, T], BF16, isOutput=False)
    w_d = {
        "wq": nc.declare_dram_parameter(
            "wq", [P, KD * PAIRS * H2], BF16, isOutput=False
        ),
        "wk": nc.declare_dram_parameter(
            "wk", [P, KD * PAIRS * H2], BF16, isOutput=False
        ),
        "wv": nc.declare_dram_parameter(
            "wv", [P, KD * HPB * DH], BF16, isOutput=False
        ),
    }
    out_d = nc.declare_dram_parameter(
        "out", [PAIRS, 2, DH + 1, T], F32, isOutput=True
    )
    with tile.TileContext(nc) as tc:
        _build(nc, tc, xT_d, w_d, out_d)
    nc.compile()
    return nc


_CACHE = {}


def _get_nc():
    if "nc" not in _CACHE:
        _CACHE["nc"] = build_bass()
    return _CACHE["nc"]


def make_in_maps(x, W_K, W_Q, W_V):
    x = np.asarray(x, dtype=np.float32)
    in_maps = []
    for c in range(NCORES):
        b = c // 2
        hb = (c % 2) * HPB
        xT = np.ascontiguousarray(x[b].T).astype(ml_dtypes.bfloat16)

        def stat(w):  # stationary layout for Q/K: [P, PAIRS, KD, H2]
            w = np.asarray(w, dtype=np.float32)
            arr = np.empty((P, PAIRS, KD, H2), np.float32)
            for p in range(PAIRS):
                wp = w[hb + 2 * p : hb + 2 * p + 2].reshape(H2, D).T  # [D, H2]
                arr[:, p, :, :] = wp.reshape(KD, P, H2).transpose(1, 0, 2)
            return np.ascontiguousarray(
                arr.reshape(P, PAIRS * KD * H2)
            ).astype(ml_dtypes.bfloat16)

        def mov(w):  # moving layout for V: [P, KD, HPB*DH]
            w = np.asarray(w, dtype=np.float32)
            wt = w[hb : hb + HPB].reshape(HPB * DH, D).T  # [D, 8*64] head-major
            wt = wt.reshape(KD, P, HPB * DH).transpose(1, 0, 2)
            return np.ascontiguousarray(
                wt.reshape(P, KD * HPB * DH)
            ).astype(ml_dtypes.bfloat16)

        in_maps.append(
            {"xT": xT, "wq": stat(W_Q), "wk": stat(W_K), "wv": mov(W_V)}
        )
    return in_maps


def kernel(x, W_K, W_Q, W_V, _trace=False, _trace_kwargs=None):
    in_maps = make_in_maps(x, W_K, W_Q, W_V)
    res = run_bass_kernel_spmd(
        _get_nc(),
        in_maps,
        list(range(NCORES)),
        trace=_trace,
        **(_trace_kwargs or {}),
    )
    _CACHE["last_results"] = res
    out = np.empty((B, T, NH * DH), np.float32)
    for c in range(NCORES):
        zt = np.asarray(res.results[c]["out"])  # [PAIRS, 2, DH+1, T]
        z = zt[:, :, :DH, :] / zt[:, :, DH : DH + 1, :]
        b = c // 2
        hb = (c % 2) * HPB
        for p in range(PAIRS):
            for a in range(2):
                h = hb + 2 * p + a
                out[b, :, h * DH : (h + 1) * DH] = z[p, a].T
    return out
